# revision 20
# baseline (speedup 1.0000x reference)
"""NetVLAD aggregator (B=16, C=2048, N=1024, K=64) on 8 Trainium2 NeuronCores.

Data-parallel over the batch dim: each core processes 2 batches end-to-end
(conv weight + centroids replicated), so no collectives are needed.

Per-core pipeline (per batch, in two n-halves of 512 for DMA/compute overlap):
  1. DMA x half  -> SBUF as [128p(c), 16cb, 512n]
  2. mm1: logits[k, n] = wT.T @ x          (PE, contraction over c)
  3. exp on ACT (PSUM->SBUF, fused cast), PE-transpose to [n, k] layout,
     softmax over k in the free dim (DVE)
  4. PE-transpose x -> xT [128p(n), c] (PSUM), evacuate via DVE/ACT copies
  5. mm2: ax[k, c] += aT.T @ xT            (PE, contraction over n)
     plus a_sum[k]  += aT.T @ ones
  6. epilogue (both batches stacked on 128 partitions):
     vlad = ax - a_sum * centroids; global L2 norm per batch; DMA out.
"""

import numpy as np

K = 64
C = 2048
N = 1024
B = 16
NCORES = 8
BPC = B // NCORES  # batches per core
NH = 2  # n-halves per batch (pipeline granularity)
NHALF = N // NH  # 512
CB = C // 128  # 16 c-blocks
NB_H = NHALF // 128  # 4 n-blocks per half
CC = C // 512  # 4 c-chunks of 512 (PSUM bank granularity)

# Matmul/transpose operand dtype: "float32" (exact, slow PE),
# "float32r" (single-pass fp32 PE mode), "bfloat16" (fastest).
DTYPE_MODE = "float32r"

_RUNNERS = {}


def _build(mode):
    import concourse.bass as bass
    import concourse.mybir as mybir
    import concourse.tile as tile
    from concourse import bacc
    from concourse.masks import make_identity

    dt = mybir.dt
    f32 = dt.float32
    # Matmul-operand tile dtype. The walrus verifier requires every operand
    # of a float32r matmul to be produced by an instruction whose output
    # dtype is float32r — DMA, DVE copies, and ACT activations all qualify
    # (memset/affine_select do not, so constants are built in fp32 and
    # DVE-copied into their f32r tiles).
    D = {
        "float32": f32,
        "float32r": dt.float32r,
        "bfloat16": dt.bfloat16,
    }[mode]

    def mc(ap):
        return ap

    AF = mybir.ActivationFunctionType

    nc = bacc.Bacc()

    # bf16 needs an SWDGE cast during the inbound DMA, so DRAM stays fp32
    # there; float32r is bit-compatible with fp32 so declaring the x DRAM
    # tensor f32r keeps the load a same-dtype HWDGE copy.
    x_dram_dt = f32 if mode == "bfloat16" else D
    x_in = nc.dram_tensor("x", [BPC, C, N], x_dram_dt, kind="ExternalInput")
    w_in = nc.dram_tensor("conv_w", [K, C], f32, kind="ExternalInput")
    cent_in = nc.dram_tensor("centroids", [K, C], f32, kind="ExternalInput")
    y_out = nc.dram_tensor("y", [BPC, K * C], f32, kind="ExternalOutput")

    with tile.TileContext(nc) as tc:
        with (
            tc.tile_pool(name="setup", bufs=1) as setup,
            tc.tile_pool(name="xpool", bufs=2) as xpool,
            tc.tile_pool(name="xtpool", bufs=2) as xtpool,
            tc.tile_pool(name="spool", bufs=2) as spool,
            tc.tile_pool(name="epool", bufs=2) as epool,
            tc.tile_pool(name="ps", bufs=1, space="PSUM") as ps,
            tc.tile_pool(name="psxt", bufs=2, space="PSUM") as psxt,
        ):
            # --- constants / weights setup ---
            ident32 = setup.tile([128, 128], f32)
            make_identity(nc, ident32)
            ones_kk = setup.tile([128, K], f32)
            nc.vector.memset(ones_kk, 1.0)
            if D != f32:
                identD = setup.tile([128, 128], D)
                nc.vector.tensor_copy(identD, ident32)
                ones_n = setup.tile([128, 2], D)
                nc.vector.tensor_copy(ones_n, ones_kk[:, 0:2])
            else:
                identD = ident32
                ones_n = setup.tile([128, 2], f32)
                nc.vector.memset(ones_n, 1.0)

            cent_sb = setup.tile([K, C], f32)
            nc.sync.dma_start(out=cent_sb, in_=cent_in[:, :])

            # wT[c, k] = conv_w[k, c] via PE transposes of the fp32 weight
            w_sb = setup.tile([K, C], f32)
            nc.sync.dma_start(out=w_sb, in_=w_in[:, :])
            wT = setup.tile([128, CB, K], D)
            for g in range(CB // 4):
                wt_ps = psxt.tile([128, 4, K], f32, tag="xt")
                for j in range(4):
                    cb = g * 4 + j
                    nc.tensor.transpose(
                        wt_ps[:, j, :],
                        w_sb[:, cb * 128 : (cb + 1) * 128],
                        ident32[:K, :K],
                    )
                nc.vector.tensor_copy(wT[:, g * 4 : (g + 1) * 4, :], wt_ps)

            # per-batch sum-of-squares columns, consumed by the joint norm
            ss2 = setup.tile([K, BPC], f32)

            x_re = [
                x_in[b].rearrange("(cb p) n -> p cb n", p=128) for b in range(BPC)
            ]
            vlads = []

            for b in range(BPC):
                # fp32r matmuls cannot target a col-group (partition-base 64)
                # PSUM tile, so each batch accumulates at partition base 0 and
                # is flushed to SBUF in its own epilogue below.
                ax_ps = ps.tile([K, C], f32, tag="ax")
                asum_ps = ps.tile([K, 2], f32, tag="sm")
                for h in range(NH):
                    first = h == 0
                    last = h == NH - 1
                    nsl = slice(h * NHALF, (h + 1) * NHALF)

                    # ---- load x half ----
                    xh = xpool.tile([128, CB, NHALF], D, tag="x")
                    for g in range(4):
                        gs = slice(g * 4, (g + 1) * 4)
                        if mode == "bfloat16":
                            nc.gpsimd.dma_start(
                                out=xh[:, gs, :], in_=x_re[b][:, gs, nsl]
                            )
                        else:
                            nc.sync.dma_start(
                                out=xh[:, gs, :], in_=x_re[b][:, gs, nsl]
                            )

                    # ---- mm1: logits over this n-half ----
                    l_ps = ps.tile([K, NHALF], f32, tag="let")
                    for cb in range(CB):
                        nc.tensor.matmul(
                            l_ps,
                            mc(wT[:, cb, :]),
                            mc(xh[:, cb, :]),
                            start=(cb == 0),
                            stop=(cb == CB - 1),
                        )

                    # ---- softmax over k (free dim after transpose) ----
                    e_sb = spool.tile([K, NHALF], D, tag="e")
                    nc.scalar.activation(e_sb, l_ps, AF.Exp)

                    et_ps = ps.tile([128, NB_H, K], D, tag="let")
                    for nb in range(NB_H):
                        nc.tensor.transpose(
                            mc(et_ps[:, nb, :]),
                            mc(e_sb[:, nb * 128 : (nb + 1) * 128]),
                            mc(identD[:K, :K]),
                        )
                    et_sb = spool.tile([128, NB_H, K], D, tag="et")
                    nc.vector.tensor_copy(et_sb, et_ps)

                    s_sb = spool.tile([128, NB_H], f32, tag="s")
                    nc.vector.reduce_sum(s_sb, et_sb, axis=mybir.AxisListType.X)
                    r_sb = spool.tile([128, NB_H], f32, tag="r")
                    nc.vector.reciprocal(r_sb, s_sb)
                    aT_sb = spool.tile([128, NB_H, K], D, tag="aT")
                    for nb in range(NB_H):
                        nc.vector.tensor_scalar_mul(
                            aT_sb[:, nb, :], et_sb[:, nb, :], r_sb[:, nb : nb + 1]
                        )

                    # ---- transpose x, then mm2 accumulation ----
                    xt_sb = xtpool.tile([128, NB_H, C], D, tag="xt")
                    for nb in range(NB_H):
                        nbs = slice(nb * 128, (nb + 1) * 128)
                        for g in range(4):
                            xt_ps = psxt.tile([128, 4, 128], D, tag="xt")
                            for j in range(4):
                                cb = g * 4 + j
                                nc.tensor.transpose(
                                    mc(xt_ps[:, j, :]), mc(xh[:, cb, nbs]), mc(identD)
                                )
                            dst = xt_sb[:, nb, g * 512 : (g + 1) * 512].rearrange(
                                "p (a c) -> p a c", a=4
                            )
                            if g % 2 == 0:
                                nc.vector.tensor_copy(dst, xt_ps)
                            else:
                                nc.scalar.copy(out=dst, in_=xt_ps)

                        for cc in range(CC):
                            csl = slice(cc * 512, (cc + 1) * 512)
                            nc.tensor.matmul(
                                ax_ps[:, csl],
                                mc(aT_sb[:, nb, :]),
                                mc(xt_sb[:, nb, csl]),
                                start=(first and nb == 0),
                                stop=(last and nb == NB_H - 1),
                                skip_group_check=True,
                            )
                        nc.tensor.matmul(
                            asum_ps,
                            mc(aT_sb[:, nb, :]),
                            mc(ones_n[:]),
                            start=(first and nb == 0),
                            stop=(last and nb == NB_H - 1),
                            skip_group_check=True,
                        )

                # ---- per-batch epilogue: vlad_b = ax - a_sum*centroids ----
                asum_sb = epool.tile([K, 1], f32, tag="asum")
                nc.vector.tensor_scalar_mul(asum_sb, asum_ps[:, 0:1], -1.0)
                cmul = epool.tile([K, C], f32, tag="cmul")
                nc.vector.tensor_scalar_mul(cmul, cent_sb, asum_sb)
                vlad = epool.tile([K, C], f32, tag="vlad")
                nc.vector.tensor_add(vlad, ax_ps, cmul)
                nc.scalar.activation(
                    cmul, vlad, AF.Square, accum_out=ss2[:, b : b + 1]
                )
                vlads.append(vlad)

            # ---- joint L2 norms (one matmul sums ss over k), scale, store ----
            tot_ps = ps.tile([K, BPC], f32, tag="sm")
            nc.tensor.matmul(
                tot_ps,
                ones_kk[:K, :],
                ss2,
                start=True,
                stop=True,
                skip_group_check=True,
            )
            norm = setup.tile([K, BPC], f32)
            nc.scalar.activation(norm, tot_ps, AF.Sqrt)
            nc.vector.tensor_scalar_max(norm, norm, 1e-12)
            rinv = setup.tile([K, BPC], f32)
            nc.vector.reciprocal(rinv, norm)
            for b in range(BPC):
                nc.vector.tensor_scalar_mul(vlads[b], vlads[b], rinv[:, b : b + 1])
                nc.sync.dma_start(
                    out=y_out[b].rearrange("(k c) -> k c", c=C),
                    in_=vlads[b],
                )

    nc.finalize()
    return nc


class _Runner:
    """Persistent compiled SPMD runner (mirrors bass2jax.run_bass_via_pjrt's
    multi-core path, but caches the jitted executable and skips donation so
    repeated timed invocations are possible)."""

    def __init__(self, mode):
        import jax
        import concourse.mybir as mybir
        from concourse.bass2jax import (
            _bass_exec_p,
            install_neuronx_cc_hook,
            partition_id_tensor,
        )
        from jax.sharding import Mesh, PartitionSpec
        from jax.experimental.shard_map import shard_map

        install_neuronx_cc_hook()
        self.jax = jax
        nc = _build(mode)
        self.nc = nc

        partition_name = (
            nc.partition_id_tensor.name if nc.partition_id_tensor else None
        )
        in_names, out_names, out_avals = [], [], []
        for alloc in nc.m.functions[0].allocations:
            if not isinstance(alloc, mybir.MemoryLocationSet):
                continue
            name = alloc.memorylocations[0].name
            if alloc.kind == "ExternalInput":
                if name != partition_name:
                    in_names.append(name)
            elif alloc.kind == "ExternalOutput":
                out_names.append(name)
                out_avals.append(
                    jax.core.ShapedArray(
                        tuple(alloc.tensor_shape), mybir.dt.np(alloc.dtype)
                    )
                )
        self.in_names = list(in_names)
        self.out_names = list(out_names)
        self.out_avals = out_avals
        all_in_names = in_names + out_names
        if partition_name is not None:
            all_in_names = all_in_names + [partition_name]

        def _body(*args):
            operands = list(args)
            if partition_name is not None:
                operands.append(partition_id_tensor())
            outs = _bass_exec_p.bind(
                *operands,
                out_avals=tuple(out_avals),
                in_names=tuple(all_in_names),
                out_names=tuple(out_names),
                lowering_input_output_aliases=(),
                sim_require_finite=False,
                sim_require_nnan=False,
                nc=nc,
            )
            return tuple(outs)

        devices = jax.devices()[:NCORES]
        assert len(devices) == NCORES, f"need {NCORES} cores, got {len(devices)}"
        self.mesh = Mesh(np.asarray(devices), ("core",))
        n_ops = len(in_names) + len(out_names)
        self._jit = jax.jit(
            shard_map(
                _body,
                mesh=self.mesh,
                in_specs=(PartitionSpec("core"),) * n_ops,
                out_specs=(PartitionSpec("core"),) * len(out_names),
                check_rep=False,
            )
        )

    def prepare(self, x, conv_w, centroids):
        """Build the global (concatenated-over-cores) operand list."""
        x = np.ascontiguousarray(np.asarray(x, dtype=np.float32))
        conv_w = np.ascontiguousarray(np.asarray(conv_w, dtype=np.float32))
        centroids = np.ascontiguousarray(np.asarray(centroids, dtype=np.float32))
        per = {
            "x": x.reshape(NCORES * BPC, C, N),  # axis0 = core*BPC
            "conv_w": np.concatenate([conv_w] * NCORES, axis=0),
            "centroids": np.concatenate([centroids] * NCORES, axis=0),
        }
        ops = [per[name] for name in self.in_names]
        ops.append(np.zeros((NCORES * BPC, K * C), np.float32))  # y zero-init
        return ops

    def run_global(self, ops):
        outs = self._jit(*ops)
        return [np.asarray(o) for o in outs]

    def __call__(self, x, conv_w, centroids):
        ops = self.prepare(x, conv_w, centroids)
        outs = self.run_global(ops)
        y = outs[self.out_names.index("y")]
        return y.reshape(B, K * C)


def get_runner(mode=None):
    mode = mode or DTYPE_MODE
    if mode not in _RUNNERS:
        _RUNNERS[mode] = _Runner(mode)
    return _RUNNERS[mode]


def kernel(x, conv_w, centroids):
    out = get_runner()(x, conv_w, centroids)
    return out.astype(np.float32, copy=False)
